# revision 44
# baseline (speedup 1.0000x reference)
"""ComplEx forward v3: split DMA queues + tail-chunk trim + prologue pipelining.

Differences from v2:
  - Entity columns are processed as 24 chunks of 512 + 1 tail chunk of 256
    (12544 computed vs 12800 before; 12500 real). PE row count drops ~2%.
  - u8 output DMAs issue from the ACT (scalar) queue while ent-prefetch DMAs
    stay on the SP (sync) queue.  With both on SP, the out-DMA (whose sem
    wait -- quant done -- is evaluated while HOLDING the sequencer) head-of-
    line blocked the next chunk's ent prefetch.  TRN2 has two HWDGE rings
    (qSPDynamicHW / qActDynamicHW), so the split maps onto real hardware.
  - ent pool deepened to 4 buffers (prefetch 3 chunks ahead).
  - Prologue: host pre-doubles gamma/beta (folds the 2*rel algebra into the
    BN affine), e1 DMA is split per k-tile so BN starts after 0.5 MB, and
    the affine runs on the ACT engine, leaving DVE only stats + combine.
  - im path dropped as in v2: combined == bn0(e1) * (2*rel) exactly.
  - Sigmoid reads 4 PSUM banks per ACT op, writes fp16; DVE casts fp16 ->
    uint8 (round(255*y)); host de-quantizes by /255.
"""

import os
import sys

if "/opt/trn_rl_repo" not in sys.path:
    sys.path.insert(0, "/opt/trn_rl_repo")

# fp16 operand mode (halves ent DMA + PE moving-port traffic; larger quant err)
F16_MODE = os.environ.get("KERNEL_F16", "0") == "1"
# fp16 sigmoid output (skips the DVE u8 quant stage; host casts to f32)
F16OUT_MODE = os.environ.get("KERNEL_F16OUT", "0") == "1"
OUTQ = os.environ.get("KERNEL_OUTQ", "act")
PSUM_BT = int(os.environ.get("KERNEL_PSUMBT", "4"))
ENT_GROUP = int(os.environ.get("KERNEL_ENTGROUP", "1"))

import numpy as np
from contextlib import ExitStack

import concourse.bass as bass
import concourse.tile as tile
from concourse import bacc, mybir
from concourse.bass_utils import run_bass_kernel_spmd

P = 128
B = 1024
D = 512
KT = D // P  # 4
BT = B // P  # 8
N_TOTAL = 100000
NCORES = 8
N_REAL = N_TOTAL // NCORES  # 12500
NCHUNK = 512
FULL_CHUNKS = 24
TAIL = 256  # 24*512 + 256 = 12544 >= 12500
N_SLAB = FULL_CHUNKS * NCHUNK + TAIL  # 12544
BN_EPS = 1e-5
HALF = 4  # bt tiles per PSUM mega-tile

F32 = mybir.dt.float32
F32R = mybir.dt.float32r
F16 = mybir.dt.float16
BF16 = mybir.dt.bfloat16
U8 = mybir.dt.uint8

_CACHE = {}


def _build(repeat=1, drop=(), hwloop=1, outq="act", ent_bufs=4, psum_bt=HALF,
           ent_group=1, f16=False, f16out=False):
    """drop: subset of {"mm","act","quant","outdma","indma"} plus diagnostic
    stream variants {"sharedw","bf16mm"} for ablations.
    hwloop: wrap the main chunk loop in a hardware For_i loop running it
    `hwloop` times — measurement amplification at constant compile cost.
    outq: engine queue for output DMAs ("sp"|"act"|"pool").
    psum_bt: bt-tiles per PSUM tile (4 -> 2 tiles of 4 banks; 2 -> 4 tiles
    of 2 banks; 1 -> 8 single-bank tiles)."""
    nc = bacc.Bacc(None, target_bir_lowering=False)

    # All inputs staged d-major by the host. gbT carries 2*gamma0 / 2*beta0.
    e1T = nc.dram_tensor("e1T", [D, B], F32, kind="ExternalInput")
    relT = nc.dram_tensor("relT", [D, B], F32, kind="ExternalInput")
    entT = nc.dram_tensor("entT", [D, N_SLAB], F16 if f16 else F32R,
                          kind="ExternalInput")
    gbT = nc.dram_tensor("gbT", [D, 2], F32, kind="ExternalInput")
    out = nc.dram_tensor("out", [B, N_SLAB], F16 if f16out else U8,
                         kind="ExternalOutput")
    entT16 = None
    if "ent16dma" in drop:
        entT16 = nc.dram_tensor("entT16", [D, N_SLAB], BF16, kind="ExternalInput")

    out_pbn = out[:].rearrange("(bt p) n -> p bt n", p=P)
    entT_pkn = (
        entT16[:].rearrange("(kt p) n -> p kt n", p=P)
        if entT16 is not None
        else entT[:].rearrange("(kt p) n -> p kt n", p=P)
    )
    e1T_pkb = e1T[:].rearrange("(kt p) b -> p kt b", p=P)

    with tile.TileContext(nc) as tc:
        with ExitStack() as ctx:
            const = ctx.enter_context(tc.tile_pool(name="const", bufs=1))
            big = ctx.enter_context(tc.tile_pool(name="big", bufs=1))
            ent_p = ctx.enter_context(tc.tile_pool(name="ent", bufs=ent_bufs))
            y16_p = ctx.enter_context(
                tc.tile_pool(name="y16", bufs=(8 // psum_bt) + 1)
            )
            u8_p = ctx.enter_context(tc.tile_pool(name="u8", bufs=3))
            pso = ctx.enter_context(
                tc.tile_pool(name="pso", bufs=8 // psum_bt, space="PSUM")
            )

            eps_t = const.tile([P, 1], F32)
            nc.vector.memset(eps_t, BN_EPS)

            # ---------------- prologue: BN + combine ----------------
            e1_sb = big.tile([P, KT, B], F32, tag="e1_sb")
            rel_sb = big.tile([P, KT, B], F32, tag="rel_sb")
            gb_sb = const.tile([P, KT, 2], F32)
            combT = big.tile([P, KT, B], F16 if f16 else F32R, tag="combT")
            nc.sync.dma_start(gb_sb[:], gbT[:].rearrange("(kt p) c -> p kt c", p=P))
            # per-k e1 DMA so BN k=0 starts after 0.5 MB, not 2 MB
            for k in range(KT):
                nc.sync.dma_start(e1_sb[:, k, :], e1T_pkb[:, k, :])
            nc.sync.dma_start(rel_sb[:], relT[:].rearrange("(kt p) b -> p kt b", p=P))

            for k in range(KT):
                # BN stats over the batch (free axis), 2 subgroups of 512
                stats = const.tile([P, 2, 6], F32, tag="bnstats")
                xk = e1_sb[:, k, :].rearrange("p (s f) -> p s f", s=2)
                nc.vector.bn_stats(stats[:, 0, :], xk[:, 0, :])
                nc.vector.bn_stats(stats[:, 1, :], xk[:, 1, :])
                mv = const.tile([P, 2], F32, tag="bnmv")
                nc.vector.bn_aggr(mv[:], stats[:])
                mean = mv[:, 0:1]
                var = mv[:, 1:2]
                rstd = const.tile([P, 1], F32, tag="rstd")
                nc.scalar.activation(
                    rstd[:], var, mybir.ActivationFunctionType.Sqrt, bias=eps_t[:]
                )
                nc.vector.reciprocal(rstd[:], rstd[:])

                # re2 = e1*a0 + b0' with a0 = rstd*(2 gamma0),
                # b0' = 2 beta0 - mean*a0  (the 2x from re_rel+im_rel folded in)
                a0 = const.tile([P, 1], F32, tag="a0")
                b0p = const.tile([P, 1], F32, tag="b0p")
                nc.vector.tensor_mul(a0[:], rstd[:], gb_sb[:, k, 0:1])
                nc.vector.tensor_mul(b0p[:], mean, a0[:])
                nc.vector.tensor_tensor(
                    b0p[:], gb_sb[:, k, 1:2], b0p[:], mybir.AluOpType.subtract
                )
                # affine on ACT (frees DVE), combine on DVE
                nc.scalar.activation(
                    e1_sb[:, k, :],
                    e1_sb[:, k, :],
                    mybir.ActivationFunctionType.Identity,
                    bias=b0p[:],
                    scale=a0[:],
                )
                nc.vector.tensor_mul(combT[:, k, :], e1_sb[:, k, :], rel_sb[:, k, :])

            combT16 = None
            if "bf16mm" in drop:
                combT16 = big.tile([P, KT, B], BF16, tag="combT16")
                for k in range(KT):
                    nc.vector.tensor_copy(combT16[:, k, :], combT[:, k, :])
            entR = None
            if "indma" in drop and "mm" not in drop:
                # resident rhs tile distinct from combT: clean-stream ablation
                entR = big.tile([P, KT, NCHUNK], F16 if f16 else F32R, tag="entR")
                nc.sync.dma_start(entR[:], entT_pkn[:, :, 0:NCHUNK])

            # ---------------- main loop over entity chunks ----------------
            import contextlib

            outer = tc.For_i(0, hwloop, 1) if hwloop > 1 else contextlib.nullcontext()
            with outer:
                _main_loop(nc, tc, repeat, drop, combT, entT_pkn, out_pbn,
                           ent_p, y16_p, u8_p, pso, outq, psum_bt, combT16,
                           ent_group, f16, entR, f16out)

    nc.compile()
    return nc


def _main_loop(nc, tc, repeat, drop, combT, entT_pkn, out_pbn,
               ent_p, y16_p, u8_p, pso, outq, psum_bt=HALF, combT16=None,
               ent_group=1, f16=False, entR=None, f16out=False):
    out_eng = {"sp": nc.sync, "act": nc.scalar, "pool": nc.gpsimd}[outq]
    chunk_cols = [NCHUNK] * FULL_CHUNKS + [TAIL]
    chunk_off = [0] * len(chunk_cols)
    for i in range(1, len(chunk_cols)):
        chunk_off[i] = chunk_off[i - 1] + chunk_cols[i - 1]
    G = psum_bt
    groups = BT // G
    ent_dt = BF16 if "ent16dma" in drop else (F16 if f16 else F32R)

    # map chunk index -> (group tile spec); groups batch `ent_group`
    # consecutive equal-width chunks into one DMA (bigger descriptors,
    # fewer DMA events)
    dma_plan = {}  # ci -> (gcols, goff, n_sub)
    sub_of = {}
    i = 0
    while i < len(chunk_cols):
        n = 1
        while (n < ent_group and i + n < len(chunk_cols)
               and chunk_cols[i + n] == chunk_cols[i]):
            n += 1
        dma_plan[i] = (chunk_cols[i] * n, chunk_off[i], n)
        for j in range(n):
            sub_of[i + j] = (i, j)
        i += n

    ent_tiles = {}
    for ci in [c for _ in range(repeat) for c in range(len(chunk_cols))]:
        cols = chunk_cols[ci]
        off = chunk_off[ci]
        lead, sub = sub_of[ci]
        if "indma" not in drop:
            if sub == 0:
                gcols, goff, n_sub = dma_plan[lead]
                gt = ent_p.tile([P, KT, gcols], ent_dt, tag=f"ent_sb{gcols}")
                nc.sync.dma_start(gt[:], entT_pkn[:, :, goff:goff + gcols])
                ent_tiles[lead] = gt
            ent_sb = ent_tiles[lead][:, :, sub * cols:(sub + 1) * cols]
        else:
            ent_sb = None  # ablation: use combT slices instead

        if f16out:
            u8c = y16_p.tile([P, BT, cols], F16, tag=f"y16c{cols}")
        else:
            u8c = u8_p.tile([P, BT, cols], U8, tag=f"u8c{cols}")
        for g in range(groups):
            ps = pso.tile([P, G, NCHUNK], F32, tag="pso")
            if "mm" not in drop:
                for bti in range(G):
                    bt = g * G + bti
                    for k in range(KT):
                        if "bf16mm" in drop:
                            lhsT = combT16[:, 0, 0:P] if "sharedw" in drop \
                                else combT16[:, k, bt * P:(bt + 1) * P]
                            rhs = (
                                ent_sb[:, k, :]
                                if (ent_sb is not None and "ent16dma" in drop)
                                else combT16[:, k, 0:cols]
                            )
                        else:
                            lhsT = combT[:, 0, 0:P] if "sharedw" in drop \
                                else combT[:, k, bt * P:(bt + 1) * P]
                            if ent_sb is not None:
                                rhs = ent_sb[:, k, :]
                            elif entR is not None:
                                rhs = entR[:, k, 0:cols]
                            else:
                                rhs = combT[:, k, :cols]
                        nc.tensor.matmul(
                            ps[:, bti, 0:cols],
                            lhsT,
                            rhs,
                            start=(k == 0),
                            stop=(k == KT - 1),
                        )
            if f16out:
                # sigmoid straight into the per-chunk fp16 output tile;
                # no DVE quant stage at all
                nc.scalar.activation(
                    u8c[:, g * G:(g + 1) * G, :],
                    ps[:, :, 0:cols],
                    mybir.ActivationFunctionType.Sigmoid,
                )
                continue
            y16 = y16_p.tile([P, G, cols], F16, tag=f"y16{cols}")
            if "act" not in drop:
                nc.scalar.activation(
                    y16[:], ps[:, :, 0:cols], mybir.ActivationFunctionType.Sigmoid
                )
            elif "mm" in drop:
                # ps never written: full memset so quant has defined input
                nc.vector.memset(y16[:], 0.5)
            else:
                nc.vector.memset(y16[:, 0, 0:1], 0.5)
            if "quant" not in drop:
                # DVE's float->uint8 cast rounds to nearest, so a bare
                # *255 yields round(255*y).
                nc.vector.tensor_scalar_mul(
                    u8c[:, g * G:(g + 1) * G, :],
                    y16[:],
                    255.0,
                )
            else:
                nc.vector.memset(u8c[:, g * G, 0:1], 1)
        if "outdma" not in drop:
            out_eng.dma_start(out_pbn[:, :, off:off + cols], u8c[:])


def _get_nc(repeat=1, drop=(), hwloop=1, outq=None, ent_bufs=4, psum_bt=None,
            ent_group=None, f16=None, f16out=None):
    if f16 is None:
        f16 = F16_MODE
    if f16out is None:
        f16out = F16OUT_MODE
    if outq is None:
        outq = OUTQ
    if psum_bt is None:
        psum_bt = PSUM_BT
    if ent_group is None:
        ent_group = ENT_GROUP
    key = (f"v3_{repeat}_{sorted(drop)}_{hwloop}_{outq}_{ent_bufs}_{psum_bt}"
           f"_{ent_group}_{f16}_{f16out}")
    if key not in _CACHE:
        _CACHE[key] = _build(repeat, drop=drop, hwloop=hwloop, outq=outq,
                             ent_bufs=ent_bufs, psum_bt=psum_bt,
                             ent_group=ent_group, f16=f16, f16out=f16out)
    return _CACHE[key]


def _prep_per_core(inputs):
    e1T = np.ascontiguousarray(
        np.asarray(inputs["e1_emb"], dtype=np.float32).T
    )
    relT = np.ascontiguousarray(
        np.asarray(inputs["rel_emb"], dtype=np.float32).T
    )
    ent = np.asarray(inputs["all_ent_emb"], dtype=np.float32)
    gbT = np.ascontiguousarray(
        np.stack(
            [
                2.0 * np.asarray(inputs["gamma0"], dtype=np.float32),
                2.0 * np.asarray(inputs["beta0"], dtype=np.float32),
            ],
            axis=1,
        )
    )
    ent_np_dt = np.float16 if F16_MODE else np.float32
    per_core = []
    for c in range(NCORES):
        slab = np.zeros((D, N_SLAB), dtype=ent_np_dt)
        slab[:, :N_REAL] = ent[c * N_REAL:(c + 1) * N_REAL].T.astype(ent_np_dt)
        per_core.append({"e1T": e1T, "relT": relT, "entT": slab, "gbT": gbT})
    return per_core


def _gather(results):
    full = np.empty((B, N_TOTAL), dtype=np.float32)
    scale = np.float32(1.0 / 255.0)
    for c in range(NCORES):
        o = results[c]["out"][:, :N_REAL]
        dst = full[:, c * N_REAL:(c + 1) * N_REAL]
        if o.dtype == np.uint8:
            np.multiply(o, scale, out=dst, casting="unsafe")
        else:
            dst[...] = o.astype(np.float32)
    return full


def _run(inputs, trace=False, trace_kwargs=None):
    in_maps = _prep_per_core(inputs)
    nc = _get_nc()
    kwargs = {}
    if trace:
        kwargs["trace"] = True
        if trace_kwargs:
            kwargs.update(trace_kwargs)
    res = run_bass_kernel_spmd(nc, in_maps, core_ids=list(range(NCORES)), **kwargs)
    return _gather(res.results), res


def kernel(**inputs):
    full, _ = _run(inputs)
    return full


# ---- timing harness (same methodology as baseline kernel.py) ----

def _make_sharded(nc, n_cores=NCORES):
    import jax
    from jax.sharding import Mesh, PartitionSpec
    from jax.experimental.shard_map import shard_map
    from concourse import bass2jax as b2j

    b2j.install_neuronx_cc_hook()

    partition_name = nc.partition_id_tensor.name if nc.partition_id_tensor else None
    in_names, out_names, out_avals = [], [], []
    for alloc in nc.m.functions[0].allocations:
        if not isinstance(alloc, mybir.MemoryLocationSet):
            continue
        name = alloc.memorylocations[0].name
        if alloc.kind == "ExternalInput":
            if name != partition_name:
                in_names.append(name)
        elif alloc.kind == "ExternalOutput":
            out_names.append(name)
            shape = tuple(alloc.tensor_shape)
            dtype = mybir.dt.np(alloc.dtype)
            out_avals.append(jax.core.ShapedArray(shape, dtype))
    n_params = len(in_names)
    n_outs = len(out_avals)
    all_in_names = list(in_names) + list(out_names)
    if partition_name is not None:
        all_in_names.append(partition_name)

    donate = tuple(range(n_params, n_params + n_outs))

    def _body(*args):
        operands = list(args)
        if partition_name is not None:
            operands.append(b2j.partition_id_tensor())
        outs = b2j._bass_exec_p.bind(
            *operands,
            out_avals=tuple(out_avals),
            in_names=tuple(all_in_names),
            out_names=tuple(out_names),
            lowering_input_output_aliases=(),
            sim_require_finite=True,
            sim_require_nnan=True,
            nc=nc,
        )
        return tuple(outs)

    devices = jax.devices()[:n_cores]
    mesh = Mesh(np.asarray(devices), ("core",))
    in_specs = (PartitionSpec("core"),) * (n_params + n_outs)
    out_specs = (PartitionSpec("core"),) * n_outs
    sharded = jax.jit(
        shard_map(
            _body, mesh=mesh, in_specs=in_specs, out_specs=out_specs, check_rep=False
        ),
        donate_argnums=donate,
        keep_unused=True,
    )
    return sharded, in_names, out_names, out_avals


class _TimedRunner:
    def __init__(self, nc, per_core):
        import jax
        from jax.sharding import Mesh, NamedSharding, PartitionSpec

        self.jax = jax
        sharded, in_names, out_names, out_avals = _make_sharded(nc)
        self.sharded = sharded
        self.out_avals = out_avals
        mesh = Mesh(np.asarray(jax.devices()[:NCORES]), ("core",))
        self.shd = NamedSharding(mesh, PartitionSpec("core"))
        concat_in = [
            np.concatenate([per_core[c][nm] for c in range(NCORES)], axis=0)
            for nm in in_names
        ]
        self.dev_in = [jax.device_put(a, self.shd) for a in concat_in]
        jax.block_until_ready(self.dev_in)
        self._zeros_np = [
            np.zeros((NCORES * av.shape[0], *av.shape[1:]), av.dtype)
            for av in out_avals
        ]

    def run(self):
        import time

        jax = self.jax
        zeros = [jax.device_put(z, self.shd) for z in self._zeros_np]
        jax.block_until_ready(zeros)
        t0 = time.perf_counter()
        outs = self.sharded(*self.dev_in, *zeros)
        jax.block_until_ready(outs)
        t1 = time.perf_counter()
        for o in outs:
            o.delete()
        return (t1 - t0) * 1e9


def benchmark(inputs, iters=8, repeat=5):
    per_core = _prep_per_core(inputs)
    r1 = _TimedRunner(_get_nc(1), per_core)
    rR = _TimedRunner(_get_nc(repeat), per_core)
    for _ in range(3):
        r1.run()
        rR.run()
    t1s, tRs = [], []
    for _ in range(iters):
        t1s.append(r1.run())
        tRs.append(rR.run())
    return t1s, tRs, repeat


if __name__ == "__main__":
    rng = np.random.default_rng(0)
    ins = {
        "e1_emb": rng.standard_normal((B, D), dtype=np.float32),
        "rel_emb": rng.standard_normal((B, D), dtype=np.float32),
        "all_ent_emb": rng.standard_normal((N_TOTAL, D), dtype=np.float32),
        "gamma0": np.ones(D, np.float32),
        "beta0": np.zeros(D, np.float32),
        "gamma1": np.ones(D, np.float32),
        "beta1": np.zeros(D, np.float32),
    }
    out = kernel(**ins)
    print("out", out.shape, out.dtype, out.min(), out.max())
